# revision 9
# baseline (speedup 1.0000x reference)
"""Trainium2 Bass kernel for nn_BaseTree (decision-tree inference), v12.

Emulated-device cost model (measured): each instruction costs ~50us
fixed plus a small per-element term; DMAs cost per contiguous segment
(strided DMAs are catastrophic, contiguous ones nearly free); broadcast
(stride-0) input APs are nearly free.  So: minimize instruction count,
keep DMAs contiguous.

Algorithm (per core, pure data parallel, tree baked at build time):
  - Host passes xg[b, q] = x[b, feature[perm[q]]] where perm lays each
    heap level out in BIT-REVERSED level-local order (see below); pure
    input re-indexing done while sharding.  Thresholds get the same
    permutation.
  - comp[p, r, q] = xg > thr (broadcast threshold row), one compare per
    row-subtile (96/96/64 rows x all 255 columns, contiguous DMAs).
  - The traversal is a pure SELECT NETWORK over the level-7 block:
    positions q in the bit-reversed layout put the two children of
    position i at i (left) and i + 2^j (right), so every stage is one
    in-place copy_predicated of the upper half onto the lower half,
    predicated on that level's comparison bits.  The leaf's path bits
    are recovered WITHOUT any accumulation arithmetic: survivor position
    q encodes the path (bit_j(q) = level-j decision), so baking the
    constant 2*rev7(q) into z7 = c7 + 2*rev7(q) (one broadcast add per
    tile) makes the final surviving byte equal the leaf index exactly.
    Values <= 255, u8 exact, zero saturation.
  - Host expands value[leaf] while unsharding (a 256x8 table lookup;
    the environment's indirect DMA gather is broken).

Per core: 7 DMAs (6 x-subtiles + 1 out) + 6 compares + 2 z7-bakes +
6 per-tile selects + 2 tail-persists + 4 merged final stage ops = 28
total instructions (vs 904 for the level-by-level baseline), zero
framework scaffolding (raw Bass + _lean_init), every DMA contiguous.
Tricks: thresholds + the rev constant ride as 2 extra rows prepended
to every subtile's DMA stream (no separate const DMA); c3..c0 blocks
sit just below the z region so both tiles' last four select stages
(widths 8/4/2/1) merge into one full-width pass over a column-major
persisted tail whose survivor lands contiguous for the output DMA.
"""

import contextlib
from contextlib import ExitStack

import numpy as np

import concourse.bacc as bacc
import concourse.bass as bass_mod
import concourse.mybir as mybir
from concourse.bass_utils import run_bass_kernel_spmd

AF = mybir.AluOpType
F32 = mybir.dt.float32
U8 = mybir.dt.uint8

N_CORES = 8
P = 128
B_TOTAL = 524288
B_CORE = B_TOTAL // N_CORES      # 65536
S_CORE = B_CORE // P             # 512 rows per partition
F = 32
DEPTH = 8
N_BRANCH = 255
N_LEAF = 256
N_OUT = 8

SUBTILES = ((0, 93), (93, 186), (186, 256))
CROWS = 2                        # const rows (thr, rev) prepended per subtile
T = 2
R = S_CORE // T                  # 256 rows per partition per tile


@contextlib.contextmanager
def _lean_init():
    """Suppress Bass.__init__'s const-AP memsets + all-engine barrier.

    They cost ~12 instructions (~0.6ms here) and this kernel never uses
    const APs (no activation bias) — every dependency is explicit via
    semaphores, so the startup barrier is not needed either.
    """
    orig_memset = bass_mod.BassGpSimd.memset
    orig_barrier = bass_mod.Bass.all_engine_barrier

    class _Dummy:
        def then_inc(self, *a, **k):
            return self

        def _wait_ge(self, *a, **k):
            return self

    bass_mod.BassGpSimd.memset = lambda self, ap, constant: _Dummy()
    bass_mod.Bass.all_engine_barrier = lambda self, *a, **k: None
    try:
        yield
    finally:
        bass_mod.BassGpSimd.memset = orig_memset
        bass_mod.Bass.all_engine_barrier = orig_barrier


def _bitrev(q, bits):
    r = 0
    for _ in range(bits):
        r = (r << 1) | (q & 1)
        q >>= 1
    return r


# column offset of each level's comparison block: c6..c0 descending then z
LEVEL_OFF = {6: 0, 5: 64, 4: 96, 3: 112, 2: 120, 1: 124, 0: 126, 7: 127}


def tree_perm():
    """perm[col] = heap node id at xg column `col`: each level block (at
    LEVEL_OFF) in bit-reversed level-local order (children of position i
    at i, i+2^j).  c3..c0 sit just below the z block so the final
    select stages can run once on a persisted 31-column tail."""
    perm = np.empty(N_BRANCH, dtype=np.int64)
    for j in range(DEPTH):
        base = (1 << j) - 1
        for q in range(1 << j):
            perm[LEVEL_OFF[j] + q] = base + _bitrev(q, j)
    return perm


def build_nc(threshold_perm, rev_const, repeat=1, timing_small_input=False):
    """Build the single-core Bass program (SPMD: same program on all cores).

    threshold_perm: thresholds in xg column order.
    rev_const: [128] u8, rev_const[q] = 2 * rev7(q) for the z7 bake.
    timing_small_input: use a small xg DRAM tensor that every subtile DMA
    re-reads — identical device work per repeat with a tiny host upload
    (for wall-clock slope timing only; results are garbage).
    """
    threshold_perm = np.asarray(threshold_perm, dtype=np.float32)

    with _lean_init():
        nc = bacc.Bacc()
    # device rows per partition: each subtile = CROWS const rows + data
    SROWS = sum((r1 - r0) + CROWS for (r0, r1) in SUBTILES) * T  # 524
    xg_rows = P * (95 if timing_small_input else SROWS)
    xg = nc.dram_tensor("xg", [xg_rows, N_BRANCH], F32, kind="ExternalInput")
    out = nc.dram_tensor("out", [P, S_CORE], U8, kind="ExternalOutput")
    xv = xg[:].rearrange("(p s) n -> p s n", p=P)

    with ExitStack() as ctx:
        comp = ctx.enter_context(nc.sbuf_tensor("comp", [P, R, N_BRANCH], U8))
        xc = ctx.enter_context(nc.sbuf_tensor("xc", [P, 95, N_BRANCH], F32))
        pers = ctx.enter_context(nc.sbuf_tensor("pers", [P, 31, S_CORE], U8))
        S = ctx.enter_context(nc.semaphore("S"))
        D = ctx.enter_context(nc.semaphore("D"))
        thr_t = xc[:, 0, :]                          # [P, 255] f32 view
        rev_t = xc[:, 1, 0:32].bitcast(U8)           # [P, 128] u8 view

        n_dma = 0          # DMAs issued so far
        n_d = 0            # D value after all marked DVE ops so far
        dev_row = 0        # running device-side row offset per partition

        for rep in range(repeat):
            for t in range(T):
                lo = t * R
                for (r0, r1) in SUBTILES:
                    rw = r1 - r0
                    nrows = rw + CROWS
                    src = (
                        xv[:, :nrows, :] if timing_small_input
                        else xv[:, dev_row: dev_row + nrows, :]
                    )
                    dev_row += nrows
                    dma = nc.sync.dma_start(out=xc[:, :nrows, :], in_=src)
                    if n_d:
                        # WAR: subtile buffer still read by prev compare
                        dma._wait_ge(D, n_d)
                    dma.then_inc(S, 16)
                    n_dma += 1
                    cmp_i = nc.vector.tensor_tensor(
                        out=comp[:, r0:r1, :],
                        in0=xc[:, CROWS: CROWS + rw, :],
                        in1=thr_t.unsqueeze(1).broadcast_to(
                            [P, rw, N_BRANCH]
                        ),
                        op=AF.is_gt,
                    )
                    cmp_i._wait_ge(S, 16 * n_dma)
                    cmp_i.then_inc(D, 1)
                    n_d += 1

                # z7 = c7 + 2*rev7(q): the only arithmetic in the combine.
                # Reads rev from xc's const row -> incs D so the next
                # tile's DMA cannot overwrite xc before it runs.
                bake = nc.vector.tensor_tensor(
                    out=comp[:, :, 127:255],
                    in0=comp[:, :, 127:255],
                    in1=rev_t.unsqueeze(1).broadcast_to([P, R, 128]),
                    op=AF.add,
                )
                bake.then_inc(D, 1)
                n_d += 1
                # select network stages 6..4 (per tile, in place)
                for j in range(6, 3, -1):
                    w = 1 << j
                    off = LEVEL_OFF[j]
                    nc.vector.copy_predicated(
                        out=comp[:, :, 127: 127 + w],
                        mask=comp[:, :, off: off + w],
                        data=comp[:, :, 127 + w: 127 + 2 * w],
                    )
                # persist c3,c2,c1,c0 + z0..15 (cols 112..142) for this
                # tile, column-major so the final survivor is contiguous
                nc.vector.tensor_copy(
                    pers[:, :, lo:lo + R],
                    comp[:, :, 112:143].rearrange("p r c -> p c r"),
                )

            # merged stages 3..0 on both tiles ([P, ., S_CORE])
            if True:
                # pers rows: 0-7 = c3, 8-11 = c2, 12-13 = c1, 14 = c0,
                # 15-30 = z0..15; survivor ends at pers[:, 15, :]
                nc.vector.copy_predicated(
                    out=pers[:, 15:23, :], mask=pers[:, 0:8, :],
                    data=pers[:, 23:31, :],
                )
                nc.vector.copy_predicated(
                    out=pers[:, 15:19, :], mask=pers[:, 8:12, :],
                    data=pers[:, 19:23, :],
                )
                nc.vector.copy_predicated(
                    out=pers[:, 15:17, :], mask=pers[:, 12:14, :],
                    data=pers[:, 17:19, :],
                )
                last = nc.vector.copy_predicated(
                    out=pers[:, 15:16, :], mask=pers[:, 14:15, :],
                    data=pers[:, 16:17, :],
                )

        last.then_inc(D, 1)
        n_d += 1
        nc.sync.dma_start(out=out[:], in_=pers[:, 15, :])._wait_ge(
            D, n_d
        ).then_inc(S, 16)

    nc.compile()
    return nc


def _check_tree(cond, cond_mask):
    """Verify cond/cond_mask encode the canonical heap-ordered perfect tree."""
    n_nodes = 2 * N_LEAF - 1
    n_branch = N_LEAF - 1
    is_branch = np.zeros(n_nodes, dtype=bool)
    node_conditions = np.zeros((n_nodes, n_nodes), dtype=bool)
    node_conditions_mask = np.zeros((n_nodes, n_nodes), dtype=bool)

    stack = [(0, None)]
    while stack:
        node_id, parent_id = stack.pop()
        if parent_id is not None:
            node_conditions_mask[node_id] = node_conditions_mask[parent_id]
            node_conditions_mask[node_id][parent_id] = True
        if node_id < n_branch:
            left_id, right_id = 2 * node_id + 1, 2 * node_id + 2
            is_branch[node_id] = True
            node_conditions[left_id] = node_conditions[node_id]
            node_conditions[right_id] = node_conditions[node_id]
            node_conditions[right_id][node_id] = True
            stack.append((right_id, node_id))
            stack.append((left_id, node_id))

    leaf_ids = np.nonzero(~is_branch)[0]
    branch_ids = np.nonzero(is_branch)[0]
    c = node_conditions[np.ix_(leaf_ids, branch_ids)]
    m = node_conditions_mask[np.ix_(leaf_ids, branch_ids)]
    return np.array_equal(c, np.asarray(cond)) and np.array_equal(
        m, np.asarray(cond_mask)
    )


_NC_CACHE = {}


def kernel(x, feature, threshold, cond, cond_mask, value):
    x = np.ascontiguousarray(np.asarray(x), dtype=np.float32)
    feature = np.asarray(feature).astype(np.int64)
    threshold = np.asarray(threshold, dtype=np.float32)
    value = np.ascontiguousarray(np.asarray(value), dtype=np.float32)

    assert x.shape == (B_TOTAL, F), x.shape
    if not _check_tree(cond, cond_mask):
        raise ValueError(
            "cond/cond_mask do not encode the canonical heap-ordered tree; "
            "this kernel bakes that structure."
        )

    perm = tree_perm()
    thr_p = threshold[perm]
    rev_const = np.array([2 * _bitrev(q, 7) for q in range(128)], np.uint8)

    key = threshold.tobytes()
    if key not in _NC_CACHE:
        _NC_CACHE[key] = build_nc(thr_p, rev_const)
    nc = _NC_CACHE[key]

    xg = x[:, feature[perm]]                          # [B, 255] f32
    const0 = thr_p.astype(np.float32)                 # thr row
    const1 = np.zeros(N_BRANCH, np.float32)
    const1[:32] = np.ascontiguousarray(rev_const).view(np.float32)
    # interleave: per partition, per subtile: [thr, rev, data rows]
    xs = xg.reshape(N_CORES, P, S_CORE, N_BRANCH)
    blocks = []
    for t in range(T):
        for (r0, r1) in SUBTILES:
            blocks.append(
                np.broadcast_to(
                    np.stack([const0, const1])[None, None],
                    (N_CORES, P, 2, N_BRANCH),
                )
            )
            blocks.append(xs[:, :, t * R + r0: t * R + r1, :])
    xdev = np.ascontiguousarray(
        np.concatenate(blocks, axis=2)
    )                                                  # [8, P, 524, 255]
    shards = xdev.reshape(N_CORES, -1, N_BRANCH)
    in_maps = [{"xg": shards[i]} for i in range(N_CORES)]
    res = run_bass_kernel_spmd(nc, in_maps, list(range(N_CORES)))
    leaves = np.concatenate(
        [np.asarray(r["out"]).reshape(-1) for r in res.results]
    ).astype(np.int64)
    return value[leaves]


if __name__ == "__main__":
    import reference

    inputs = reference.setup_inputs()
    got = kernel(**{k: np.asarray(v) for k, v in inputs.items()})
    exp = np.asarray(reference.reference(**inputs))
    err = np.abs(got - exp).max()
    print("absmax err:", err)


# revision 11
# speedup vs baseline: 4.5693x; 4.5693x over previous
"""Trainium2 Bass kernel for nn_BaseTree (decision-tree inference), v13.

Emulated-device cost model (measured): each instruction costs ~50us
fixed plus a small per-element term; DMAs cost per contiguous segment
(strided DMAs are catastrophic, contiguous ones nearly free); broadcast
(stride-0) input APs are nearly free.  So: minimize instruction count,
keep DMAs contiguous.

Algorithm (per core, pure data parallel, tree baked at build time):
  - Host passes xg[b, q] = x[b, feature[perm[q]]] where perm lays each
    heap level out in BIT-REVERSED level-local order (see below); pure
    input re-indexing done while sharding.  Thresholds get the same
    permutation.
  - comp[p, r, q] = xg > thr (broadcast threshold row), one compare per
    row-subtile (96/96/64 rows x all 255 columns, contiguous DMAs).
  - The traversal is a pure SELECT NETWORK over the level-7 block:
    positions q in the bit-reversed layout put the two children of
    position i at i (left) and i + 2^j (right), so every stage is one
    in-place copy_predicated of the upper half onto the lower half,
    predicated on that level's comparison bits.  The leaf's path bits
    are recovered WITHOUT any accumulation arithmetic: survivor position
    q encodes the path (bit_j(q) = level-j decision), so baking the
    constant 2*rev7(q) into z7 = c7 + 2*rev7(q) (one broadcast add per
    tile) makes the final surviving byte equal the leaf index exactly.
    Values <= 255, u8 exact, zero saturation.
  - Host expands value[leaf] while unsharding (a 256x8 table lookup;
    the environment's indirect DMA gather is broken).

Per core: 7 DMAs (6 x-subtiles + 1 out) + 6 compares + 2 z7-bakes +
4 per-tile selects + 2 tail-persists + 5 merged final selects = 27
total instructions (vs 904 for the level-by-level baseline), zero
framework scaffolding (raw Bass + _lean_init), every DMA contiguous.
Thresholds + the rev constant ride as 2 rows prepended to each
subtile's DMA; c4..c0 sit just below the z region so both tiles' last
five select stages merge into one full-width pass over a column-major
63-column persisted tail whose survivor lands contiguous for the
output DMA (the unused dynamic-DMA scratch carveout is shrunk to fit).
"""

import contextlib
from contextlib import ExitStack

import numpy as np

import concourse.bacc as bacc
import concourse.bass as bass_mod
import concourse.mybir as mybir
from concourse.bass_utils import run_bass_kernel_spmd

AF = mybir.AluOpType
F32 = mybir.dt.float32
U8 = mybir.dt.uint8

N_CORES = 8
P = 128
B_TOTAL = 524288
B_CORE = B_TOTAL // N_CORES      # 65536
S_CORE = B_CORE // P             # 512 rows per partition
F = 32
DEPTH = 8
N_BRANCH = 255
N_LEAF = 256
N_OUT = 8

SUBTILES = ((0, 93), (93, 186), (186, 256))
CROWS = 2                        # const rows (thr, rev) prepended per subtile
T = 2
R = S_CORE // T                  # 256 rows per partition per tile


@contextlib.contextmanager
def _lean_init():
    """Suppress Bass.__init__'s const-AP memsets + all-engine barrier.

    They cost ~12 instructions (~0.6ms here) and this kernel never uses
    const APs (no activation bias) — every dependency is explicit via
    semaphores, so the startup barrier is not needed either.
    """
    orig_memset = bass_mod.BassGpSimd.memset
    orig_barrier = bass_mod.Bass.all_engine_barrier

    class _Dummy:
        def then_inc(self, *a, **k):
            return self

        def _wait_ge(self, *a, **k):
            return self

    bass_mod.BassGpSimd.memset = lambda self, ap, constant: _Dummy()
    bass_mod.Bass.all_engine_barrier = lambda self, *a, **k: None
    try:
        yield
    finally:
        bass_mod.BassGpSimd.memset = orig_memset
        bass_mod.Bass.all_engine_barrier = orig_barrier


def _bitrev(q, bits):
    r = 0
    for _ in range(bits):
        r = (r << 1) | (q & 1)
        q >>= 1
    return r


# column offset of each level's comparison block: c6..c0 descending then z
LEVEL_OFF = {6: 0, 5: 64, 4: 96, 3: 112, 2: 120, 1: 124, 0: 126, 7: 127}


def tree_perm():
    """perm[col] = heap node id at xg column `col`: each level block (at
    LEVEL_OFF) in bit-reversed level-local order (children of position i
    at i, i+2^j).  c4..c0 sit just below the z block so the final
    select stages can run once on a persisted 63-column tail."""
    perm = np.empty(N_BRANCH, dtype=np.int64)
    for j in range(DEPTH):
        base = (1 << j) - 1
        for q in range(1 << j):
            perm[LEVEL_OFF[j] + q] = base + _bitrev(q, j)
    return perm


def build_nc(threshold_perm, rev_const, repeat=1, timing_small_input=False):
    """Build the single-core Bass program (SPMD: same program on all cores).

    threshold_perm: thresholds in xg column order.
    rev_const: [128] u8, rev_const[q] = 2 * rev7(q) for the z7 bake.
    timing_small_input: use a small xg DRAM tensor that every subtile DMA
    re-reads — identical device work per repeat with a tiny host upload
    (for wall-clock slope timing only; results are garbage).
    """
    threshold_perm = np.asarray(threshold_perm, dtype=np.float32)

    with _lean_init():
        nc = bacc.Bacc(dynamic_dma_scratch_size=256)
    # device rows per partition: each subtile = CROWS const rows + data
    SROWS = sum((r1 - r0) + CROWS for (r0, r1) in SUBTILES) * T  # 524
    xg_rows = P * (95 if timing_small_input else SROWS)
    xg = nc.dram_tensor("xg", [xg_rows, N_BRANCH], F32, kind="ExternalInput")
    out = nc.dram_tensor("out", [P, S_CORE], U8, kind="ExternalOutput")
    xv = xg[:].rearrange("(p s) n -> p s n", p=P)

    with ExitStack() as ctx:
        comp = ctx.enter_context(nc.sbuf_tensor("comp", [P, R, N_BRANCH], U8))
        xc = ctx.enter_context(nc.sbuf_tensor("xc", [P, 95, N_BRANCH], F32))
        pers = ctx.enter_context(nc.sbuf_tensor("pers", [P, 63, S_CORE], U8))
        S = ctx.enter_context(nc.semaphore("S"))
        D = ctx.enter_context(nc.semaphore("D"))
        thr_t = xc[:, 0, :]                          # [P, 255] f32 view
        rev_t = xc[:, 1, 0:32].bitcast(U8)           # [P, 128] u8 view

        n_dma = 0          # DMAs issued so far
        n_d = 0            # D value after all marked DVE ops so far
        dev_row = 0        # running device-side row offset per partition

        for rep in range(repeat):
            for t in range(T):
                lo = t * R
                for (r0, r1) in SUBTILES:
                    rw = r1 - r0
                    nrows = rw + CROWS
                    src = (
                        xv[:, :nrows, :] if timing_small_input
                        else xv[:, dev_row: dev_row + nrows, :]
                    )
                    dev_row += nrows
                    dma = nc.sync.dma_start(out=xc[:, :nrows, :], in_=src)
                    if n_d:
                        # WAR: subtile buffer still read by prev compare
                        dma._wait_ge(D, n_d)
                    dma.then_inc(S, 16)
                    n_dma += 1
                    cmp_i = nc.vector.tensor_tensor(
                        out=comp[:, r0:r1, :],
                        in0=xc[:, CROWS: CROWS + rw, :],
                        in1=thr_t.unsqueeze(1).broadcast_to(
                            [P, rw, N_BRANCH]
                        ),
                        op=AF.is_gt,
                    )
                    cmp_i._wait_ge(S, 16 * n_dma)
                    cmp_i.then_inc(D, 1)
                    n_d += 1

                # z7 = c7 + 2*rev7(q): the only arithmetic in the combine.
                # Reads rev from xc's const row -> incs D so the next
                # tile's DMA cannot overwrite xc before it runs.
                bake = nc.vector.tensor_tensor(
                    out=comp[:, :, 127:255],
                    in0=comp[:, :, 127:255],
                    in1=rev_t.unsqueeze(1).broadcast_to([P, R, 128]),
                    op=AF.add,
                )
                bake.then_inc(D, 1)
                n_d += 1
                # select network stages 6..5 (per tile, in place)
                for j in range(6, 4, -1):
                    w = 1 << j
                    off = LEVEL_OFF[j]
                    nc.vector.copy_predicated(
                        out=comp[:, :, 127: 127 + w],
                        mask=comp[:, :, off: off + w],
                        data=comp[:, :, 127 + w: 127 + 2 * w],
                    )
                # persist c4..c0 + z0..31 (cols 96..158) for this tile,
                # column-major so the final survivor is contiguous
                nc.vector.tensor_copy(
                    pers[:, :, lo:lo + R],
                    comp[:, :, 96:159].rearrange("p r c -> p c r"),
                )

            # merged stages 4..0 on both tiles ([P, ., S_CORE])
            if True:
                # pers rows: 0-15 = c4, 16-23 = c3, 24-27 = c2,
                # 28-29 = c1, 30 = c0, 31-62 = z0..31
                nc.vector.copy_predicated(
                    out=pers[:, 31:47, :], mask=pers[:, 0:16, :],
                    data=pers[:, 47:63, :],
                )
                nc.vector.copy_predicated(
                    out=pers[:, 31:39, :], mask=pers[:, 16:24, :],
                    data=pers[:, 39:47, :],
                )
                nc.vector.copy_predicated(
                    out=pers[:, 31:35, :], mask=pers[:, 24:28, :],
                    data=pers[:, 35:39, :],
                )
                nc.vector.copy_predicated(
                    out=pers[:, 31:33, :], mask=pers[:, 28:30, :],
                    data=pers[:, 33:35, :],
                )
                last = nc.vector.copy_predicated(
                    out=pers[:, 31:32, :], mask=pers[:, 30:31, :],
                    data=pers[:, 32:33, :],
                )

        last.then_inc(D, 1)
        n_d += 1
        nc.sync.dma_start(out=out[:], in_=pers[:, 31, :])._wait_ge(
            D, n_d
        ).then_inc(S, 16)

    nc.compile()
    return nc


def _check_tree(cond, cond_mask):
    """Verify cond/cond_mask encode the canonical heap-ordered perfect tree."""
    n_nodes = 2 * N_LEAF - 1
    n_branch = N_LEAF - 1
    is_branch = np.zeros(n_nodes, dtype=bool)
    node_conditions = np.zeros((n_nodes, n_nodes), dtype=bool)
    node_conditions_mask = np.zeros((n_nodes, n_nodes), dtype=bool)

    stack = [(0, None)]
    while stack:
        node_id, parent_id = stack.pop()
        if parent_id is not None:
            node_conditions_mask[node_id] = node_conditions_mask[parent_id]
            node_conditions_mask[node_id][parent_id] = True
        if node_id < n_branch:
            left_id, right_id = 2 * node_id + 1, 2 * node_id + 2
            is_branch[node_id] = True
            node_conditions[left_id] = node_conditions[node_id]
            node_conditions[right_id] = node_conditions[node_id]
            node_conditions[right_id][node_id] = True
            stack.append((right_id, node_id))
            stack.append((left_id, node_id))

    leaf_ids = np.nonzero(~is_branch)[0]
    branch_ids = np.nonzero(is_branch)[0]
    c = node_conditions[np.ix_(leaf_ids, branch_ids)]
    m = node_conditions_mask[np.ix_(leaf_ids, branch_ids)]
    return np.array_equal(c, np.asarray(cond)) and np.array_equal(
        m, np.asarray(cond_mask)
    )


_NC_CACHE = {}


def kernel(x, feature, threshold, cond, cond_mask, value):
    x = np.ascontiguousarray(np.asarray(x), dtype=np.float32)
    feature = np.asarray(feature).astype(np.int64)
    threshold = np.asarray(threshold, dtype=np.float32)
    value = np.ascontiguousarray(np.asarray(value), dtype=np.float32)

    assert x.shape == (B_TOTAL, F), x.shape
    if not _check_tree(cond, cond_mask):
        raise ValueError(
            "cond/cond_mask do not encode the canonical heap-ordered tree; "
            "this kernel bakes that structure."
        )

    perm = tree_perm()
    thr_p = threshold[perm]
    rev_const = np.array([2 * _bitrev(q, 7) for q in range(128)], np.uint8)

    key = threshold.tobytes()
    if key not in _NC_CACHE:
        _NC_CACHE[key] = build_nc(thr_p, rev_const)
    nc = _NC_CACHE[key]

    xg = x[:, feature[perm]]                          # [B, 255] f32
    const0 = thr_p.astype(np.float32)                 # thr row
    const1 = np.zeros(N_BRANCH, np.float32)
    const1[:32] = np.ascontiguousarray(rev_const).view(np.float32)
    # interleave: per partition, per subtile: [thr, rev, data rows]
    xs = xg.reshape(N_CORES, P, S_CORE, N_BRANCH)
    blocks = []
    for t in range(T):
        for (r0, r1) in SUBTILES:
            blocks.append(
                np.broadcast_to(
                    np.stack([const0, const1])[None, None],
                    (N_CORES, P, 2, N_BRANCH),
                )
            )
            blocks.append(xs[:, :, t * R + r0: t * R + r1, :])
    xdev = np.ascontiguousarray(
        np.concatenate(blocks, axis=2)
    )                                                  # [8, P, 524, 255]
    shards = xdev.reshape(N_CORES, -1, N_BRANCH)
    in_maps = [{"xg": shards[i]} for i in range(N_CORES)]
    res = run_bass_kernel_spmd(nc, in_maps, list(range(N_CORES)))
    leaves = np.concatenate(
        [np.asarray(r["out"]).reshape(-1) for r in res.results]
    ).astype(np.int64)
    return value[leaves]


if __name__ == "__main__":
    import reference

    inputs = reference.setup_inputs()
    got = kernel(**{k: np.asarray(v) for k, v in inputs.items()})
    exp = np.asarray(reference.reference(**inputs))
    err = np.abs(got - exp).max()
    print("absmax err:", err)
